# revision 16
# baseline (speedup 1.0000x reference)
"""Trainium2 Bass kernel for nn_Attention_50964081935360.

Single-query attention with a global-Frobenius-norm score scale:
  scores[b,s] = key[b,s,:] . query[b,:]
  denom      = ||key||_F  (over the WHOLE key tensor, all batches)
  p          = softmax(scores/denom) masked to s < seq_lens[b], renormalized
  out        = p[..., None] + 1e-15

Sharding: data-parallel over batch B=32 across 8 NeuronCores (4 batches per
core). The only cross-core communication is one small AllReduce (sum of
squares of the key shard).

Layout: sequence position s = 32*p + t (p = SBUF partition, t = 0..31), so
 - each supertile (4 consecutive t) is ONE 2 MiB DMA with 16 KiB contiguous
   per partition (4 consecutive key rows land in one descriptor),
 - the output [128, 32] per batch DMAs straight out, no transposes.

Streaming plan (memory-bound; key shard is 64 MiB, ~200 us at ~330 GB/s):
  DVE : one affine_mul_reduce per [128,1024] tile -> scores column
  ACT : Square activations with accum_out -> ssq partial columns (PSUM out)
The last supertile uses per-tile DMAs + 1024-wide squares so the ssq total
lags the final DMA byte by only ~2 us. The local ssq is reduced on DVE
(free axis) + one gpsimd partition_all_reduce, then all-reduced as a
[128]-vector so the global sum returns already replicated per partition
(no post-collective broadcast). Dummy Ln/Exp activations right after the
last square preload the ACT tables during the collective's ~28 us latency.
Epilogue: one [128,128] Exp, 4 masked AMRs, partition_all_reduce,
reciprocal, 4 scales, one output DMA. TensorE is never used.
"""

import sys

import numpy as np

if "/opt/trn_rl_repo" not in sys.path:
    sys.path.insert(0, "/opt/trn_rl_repo")

import concourse.bacc as bacc
import concourse.bass as bass
import concourse.mybir as mybir
import concourse.tile as tile
from concourse.bass_isa import ReduceOp
from concourse.bass_utils import run_bass_kernel_spmd

B, S, D = 32, 4096, 1024
NCORES = 8
BPC = B // NCORES   # batches per core
P = 128             # partitions
TPB = S // P        # t-columns per batch (32); s = 32*p + t
SUB = 8             # t-tiles per key super-tile
NG = TPB // SUB     # super-tiles per batch (8)
NST = BPC * NG      # super-tiles per core (32)
PERTURB = 1e-15
KEY_BUFS = 5        # in-flight key super-tiles (4 MiB each)
NSQ = SUB // 2      # 2048-wide squares per bulk super-tile
NSQCOL = NSQ * (NST - 1) + SUB  # ssq partial cols (NSQ/supertile, SUB for last)

F32 = mybir.dt.float32
I32 = mybir.dt.int32
ALU = mybir.AluOpType
ACTF = mybir.ActivationFunctionType
AXL = mybir.AxisListType


def build() -> bass.Bass:
    nc = bacc.Bacc(
        "TRN2", target_bir_lowering=False, debug=False, num_devices=NCORES
    )
    key_ext = nc.declare_dram_parameter("key", [BPC, S, D], F32, isOutput=False)
    q_ext = nc.declare_dram_parameter("query", [1, BPC * D], F32, isOutput=False)
    sl_ext = nc.declare_dram_parameter("seq_lens", [1, BPC], I32, isOutput=False)
    out_ext = nc.declare_dram_parameter("out", [BPC, S, 1], F32, isOutput=True)

    # Collective bounce buffers (internal DRAM; output must be Shared).
    # Scalar-sized: ncfw latency grows badly with element count (measured
    # 28 us @ 8 elems vs 93 us @ 128 elems for AllReduce). AllGather of one
    # scalar per rank + a local 8-element sum runs one protocol phase
    # instead of AllReduce's two.
    cc_in = nc.dram_tensor("cc_in", [1, 1], F32)
    cc_out = nc.dram_tensor("cc_out", [1, NCORES], F32, addr_space="Shared")
    # Dummy collective buffers: a warm-up AllReduce at kernel start pays the
    # ncfw wakeup latency so the real one at the end doesn't.
    ccw_in = nc.dram_tensor("ccw_in", [1, 8], F32)
    ccw_out = nc.dram_tensor("ccw_out", [1, 8], F32, addr_space="Shared")

    out_ap = out_ext.ap()
    # key rows viewed as [p, t, d] with s = 32*p + t, flattened free dim.
    key_r = [
        key_ext.ap()[b].rearrange("(p t) d -> p (t d)", p=P) for b in range(BPC)
    ]

    with tile.TileContext(nc) as tc:
        with (
            tc.tile_pool(name="keys", bufs=KEY_BUFS) as kpool,
            tc.tile_pool(name="amr_scratch", bufs=4) as amrpool,
            tc.tile_pool(name="sq_psum", bufs=2, space="PSUM") as sqpool,
            tc.tile_pool(name="persist", bufs=1) as pp,
        ):
            # ---- setup: query broadcast, seq_lens, s-index, masks ----
            # q/sl ride the scalar (ACT) HWDGE ring; key loads own the sync
            # ring so they start flowing immediately.
            q_all = pp.tile([P, BPC * D], F32)
            nc.scalar.dma_start(out=q_all[0:1, :], in_=q_ext.ap()[:, :])
            sl_i = pp.tile([1, BPC], I32)
            nc.scalar.dma_start(out=sl_i[:, :], in_=sl_ext.ap()[:, :])

            # warm-up collective (result unused)
            warm = pp.tile([1, 8], F32)
            nc.vector.memset(warm[:, :], 0.0)
            nc.scalar.dma_start(out=ccw_in.ap()[:, :], in_=warm[:, :])
            nc.gpsimd.collective_compute(
                "AllReduce",
                ALU.add,
                replica_groups=[list(range(NCORES))],
                ins=[ccw_in.ap().opt()],
                outs=[ccw_out.ap().opt()],
            )

            nc.gpsimd.partition_broadcast(q_all[:, :], q_all[0:1, :])
            q_rep = [q_all[:, b * D : (b + 1) * D] for b in range(BPC)]

            sl_f = pp.tile([P, BPC], F32)
            nc.vector.tensor_copy(out=sl_f[0:1, :], in_=sl_i[:, :])
            nc.gpsimd.partition_broadcast(sl_f[:, :], sl_f[0:1, :])

            # s_idx[p, t] = 32*p + t  (the sequence position of scores[p, t])
            s_idx_i = pp.tile([P, TPB], I32)
            nc.gpsimd.iota(
                s_idx_i[:, :], pattern=[[1, TPB]], base=0, channel_multiplier=TPB
            )
            s_idx = pp.tile([P, TPB], F32)
            nc.vector.tensor_copy(out=s_idx[:, :], in_=s_idx_i[:, :])

            # masks depend only on s_idx/seq_lens: compute them up front
            masks = pp.tile([P, BPC * TPB], F32)
            for b in range(BPC):
                nc.vector.tensor_scalar(
                    out=masks[:, b * TPB : (b + 1) * TPB],
                    in0=s_idx[:, :],
                    scalar1=sl_f[:, b : b + 1],
                    scalar2=None,
                    op0=ALU.is_lt,
                )

            # ---- main streaming loop over key super-tiles ----
            scores = pp.tile([P, BPC * TPB], F32)
            ssqcols = pp.tile([P, NSQCOL], F32)

            def do_amr(kt, b, t, j):
                amr = amrpool.tile([P, D], F32, tag="amr")
                nc.vector.affine_mul_reduce(
                    out=amr[:, :],
                    accum_out=scores[:, b * TPB + t : b * TPB + t + 1],
                    in0=kt[:, j * D : (j + 1) * D],
                    in1=q_rep[b],
                    scale=1.0,
                    bias=0.0,
                )

            for b in range(BPC):
                for g in range(NG):
                    st = b * NG + g
                    kt = kpool.tile([P, SUB * D], F32, tag="key")
                    if st < NST - 1:
                        # one 4 MiB DMA; 32 KiB contiguous per partition.
                        # All key loads on the sync ring: routing them via
                        # the scalar (ACT) ring stalls issue behind the 2 us
                        # Square ops and makes the stream jagged (measured).
                        nc.sync.dma_start(
                            out=kt[:, :],
                            in_=key_r[b][:, g * SUB * D : (g + 1) * SUB * D],
                        )
                        for j in range(SUB):
                            do_amr(kt, b, g * SUB + j, j)
                        # ssq partials: square+accum, 2048 wide; out -> PSUM
                        for h in range(NSQ):
                            w = 2 * D
                            sq = sqpool.tile([P, w], F32, tag="sq")
                            nc.scalar.activation(
                                out=sq[:, :],
                                in_=kt[:, h * w : (h + 1) * w],
                                func=ACTF.Square,
                                accum_out=ssqcols[
                                    :, NSQ * st + h : NSQ * st + h + 1
                                ],
                            )
                    else:
                        # last super-tile: per-tile DMAs + 1024-wide squares
                        # so ssq lags the final DMA byte by only ~2 us
                        for j in range(SUB):
                            t = g * SUB + j
                            nc.sync.dma_start(
                                out=kt[:, j * D : (j + 1) * D],
                                in_=key_r[b][:, t * D : (t + 1) * D],
                            )
                            do_amr(kt, b, t, j)
                            sq = sqpool.tile([P, 2 * D], F32, tag="sq")
                            nc.scalar.activation(
                                out=sq[:, 0:D],
                                in_=kt[:, j * D : (j + 1) * D],
                                func=ACTF.Square,
                                accum_out=ssqcols[
                                    :, NSQ * st + j : NSQ * st + j + 1
                                ],
                            )

            # ---- dummy Exp+Ln right after the last square: forces the ACT
            # table loads to happen during the collective wait, not after.
            # Reading ssqcols' last column orders them after the last
            # accumulator read on the ACT queue. Ln LAST so the real Ln sees
            # no function-family transition (the framework reloads a table on
            # every transition, even if it was loaded before).
            dum = pp.tile([1, 2], F32)
            nc.scalar.activation(
                out=dum[:, 1:2],
                in_=ssqcols[0:1, NSQCOL - 1 : NSQCOL],
                func=ACTF.Exp,
                scale=-1e-6,
            )
            nc.scalar.activation(
                out=dum[:, 0:1], in_=ssqcols[0:1, NSQCOL - 1 : NSQCOL], func=ACTF.Ln
            )

            # ---- local ssq -> replicated scalar -> AllReduce ----
            ssq_p = pp.tile([P, 1], F32)
            nc.vector.tensor_reduce(
                out=ssq_p[:, :], in_=ssqcols[:, :], axis=AXL.X, op=ALU.add
            )
            nc.gpsimd.partition_all_reduce(
                ssq_p[:, :], ssq_p[:, :], P, ReduceOp.add
            )
            nc.scalar.dma_start(out=cc_in.ap()[:, :], in_=ssq_p[0:1, :])
            nc.gpsimd.collective_compute(
                "AllReduce",
                ALU.add,
                replica_groups=[list(range(NCORES))],
                ins=[cc_in.ap().opt()],
                outs=[cc_out.ap()[:, 0:1].opt()],
            )
            gssq = pp.tile([1, 1], F32)
            nc.scalar.dma_start(out=gssq[:, :], in_=cc_out.ap()[:, 0:1])

            # inv = 1/sqrt(gssq) = exp(-0.5 * ln(gssq)); tables preloaded.
            lng = pp.tile([1, 1], F32)
            nc.scalar.activation(out=lng[:, :], in_=gssq[:, :], func=ACTF.Ln)
            inv_rep = pp.tile([P, 1], F32)
            nc.scalar.activation(
                out=inv_rep[0:1, :], in_=lng[:, :], func=ACTF.Exp, scale=-0.5
            )
            nc.gpsimd.partition_broadcast(inv_rep[:, :], inv_rep[0:1, :])

            # ---- epilogue: masked softmax, renormalize, store ----
            e = pp.tile([P, BPC * TPB], F32)
            nc.scalar.activation(
                out=e[:, :], in_=scores[:, :], func=ACTF.Exp, scale=inv_rep[:, :]
            )
            em = pp.tile([P, BPC * TPB], F32)
            zc = pp.tile([P, BPC], F32)
            for b in range(BPC):
                sl_ = slice(b * TPB, (b + 1) * TPB)
                nc.vector.affine_mul_reduce(
                    out=em[:, sl_],
                    accum_out=zc[:, b : b + 1],
                    in0=e[:, sl_],
                    in1=masks[:, sl_],
                    scale=1.0,
                    bias=0.0,
                )
            nc.gpsimd.partition_all_reduce(zc[:, :], zc[:, :], P, ReduceOp.add)
            invz = pp.tile([P, BPC], F32)
            nc.vector.reciprocal(out=invz[:, :], in_=zc[:, :])

            o = pp.tile([P, BPC * TPB], F32)
            for b in range(BPC):
                sl_ = slice(b * TPB, (b + 1) * TPB)
                nc.vector.tensor_scalar(
                    out=o[:, sl_],
                    in0=em[:, sl_],
                    scalar1=invz[:, b : b + 1],
                    scalar2=PERTURB,
                    op0=ALU.mult,
                    op1=ALU.add,
                )
            # out[b, 32p+t] = o[p, b*32+t]: direct DMAs, 128 B per chunk,
            # alternating the two HWDGE rings so issue overlaps
            for b in range(BPC):
                dst = out_ap[b, :, 0].rearrange("(p t) -> p t", p=P)
                eng = nc.sync if b % 2 == 0 else nc.scalar
                eng.dma_start(out=dst, in_=o[:, b * TPB : (b + 1) * TPB])

    nc.compile()
    return nc


_NC_CACHE = None


def _get_nc():
    global _NC_CACHE
    if _NC_CACHE is None:
        _NC_CACHE = build()
    return _NC_CACHE


def make_in_maps(key, query, seq_lens):
    key = np.ascontiguousarray(np.asarray(key, dtype=np.float32))
    query = np.ascontiguousarray(np.asarray(query, dtype=np.float32))
    seq_lens = np.ascontiguousarray(np.asarray(seq_lens, dtype=np.int32))
    in_maps = []
    for c in range(NCORES):
        lo, hi = c * BPC, (c + 1) * BPC
        in_maps.append(
            {
                "key": key[lo:hi],
                "query": query[lo:hi].reshape(1, BPC * D),
                "seq_lens": seq_lens[lo:hi].reshape(1, BPC),
            }
        )
    return in_maps


def kernel(key, query, seq_lens, **run_kwargs):
    nc = _get_nc()
    in_maps = make_in_maps(key, query, seq_lens)
    res = run_bass_kernel_spmd(
        nc, in_maps, core_ids=list(range(NCORES)), **run_kwargs
    )
    outs = [res.results[c]["out"].reshape(BPC, S, 1) for c in range(NCORES)]
    full = np.concatenate(outs, axis=0).astype(np.float32)
    if run_kwargs:
        kernel.last_results = res  # expose profile info to test harness
    return full


# revision 17
# speedup vs baseline: 1.2758x; 1.2758x over previous
"""Trainium2 Bass kernel for nn_Attention_50964081935360.

Single-query attention with a global-Frobenius-norm score scale:
  scores[b,s] = key[b,s,:] . query[b,:]
  denom      = ||key||_F  (over the WHOLE key tensor, all batches)
  p          = softmax(scores/denom) masked to s < seq_lens[b], renormalized
  out        = p[..., None] + 1e-15

Sharding: data-parallel over batch B=32 across 8 NeuronCores (4 batches per
core). The only cross-core communication is one small AllReduce (sum of
squares of the key shard).

Layout: sequence position s = 32*p + t (p = SBUF partition, t = 0..31), so
 - each supertile (4 consecutive t) is ONE 2 MiB DMA with 16 KiB contiguous
   per partition (4 consecutive key rows land in one descriptor),
 - the output [128, 32] per batch DMAs straight out, no transposes.

Streaming plan (memory-bound; key shard is 64 MiB, ~200 us at ~330 GB/s):
  DVE : one affine_mul_reduce per [128,1024] tile -> scores column
  ACT : Square activations with accum_out -> ssq partial columns (PSUM out)
The last supertile uses per-tile DMAs + 1024-wide squares so the ssq total
lags the final DMA byte by only ~2 us. The local ssq is reduced on DVE
(free axis) + one gpsimd partition_all_reduce, then all-reduced as a
[128]-vector so the global sum returns already replicated per partition
(no post-collective broadcast). Dummy Ln/Exp activations right after the
last square preload the ACT tables during the collective's ~28 us latency.
Epilogue: one [128,128] Exp, 4 masked AMRs, partition_all_reduce,
reciprocal, 4 scales, one output DMA. TensorE is never used.
"""

import sys

import numpy as np

if "/opt/trn_rl_repo" not in sys.path:
    sys.path.insert(0, "/opt/trn_rl_repo")

import concourse.bacc as bacc
import concourse.bass as bass
import concourse.mybir as mybir
import concourse.tile as tile
from concourse.bass_isa import ReduceOp
from concourse.bass_utils import run_bass_kernel_spmd

B, S, D = 32, 4096, 1024
NCORES = 8
BPC = B // NCORES   # batches per core
P = 128             # partitions
TPB = S // P        # t-columns per batch (32); s = 32*p + t
SUB = 4             # t-tiles per key super-tile
NG = TPB // SUB     # super-tiles per batch (8)
NST = BPC * NG      # super-tiles per core (32)
PERTURB = 1e-15
KEY_BUFS = 10       # in-flight key super-tiles (2 MiB each)
NSQ = SUB // 2      # 2048-wide squares per bulk super-tile
NSQCOL = NSQ * (NST - 1) + SUB  # ssq partial cols (NSQ/supertile, SUB for last)

F32 = mybir.dt.float32
I32 = mybir.dt.int32
ALU = mybir.AluOpType
ACTF = mybir.ActivationFunctionType
AXL = mybir.AxisListType


def build() -> bass.Bass:
    nc = bacc.Bacc(
        "TRN2", target_bir_lowering=False, debug=False, num_devices=NCORES
    )
    key_ext = nc.declare_dram_parameter("key", [BPC, S, D], F32, isOutput=False)
    q_ext = nc.declare_dram_parameter("query", [1, BPC * D], F32, isOutput=False)
    sl_ext = nc.declare_dram_parameter("seq_lens", [1, BPC], I32, isOutput=False)
    out_ext = nc.declare_dram_parameter("out", [BPC, S, 1], F32, isOutput=True)

    # Collective bounce buffers (internal DRAM; output must be Shared).
    # Scalar-sized: ncfw latency grows badly with element count (measured
    # 28 us @ 8 elems vs 93 us @ 128 elems for AllReduce). AllGather of one
    # scalar per rank + a local 8-element sum runs one protocol phase
    # instead of AllReduce's two.
    cc_in = nc.dram_tensor("cc_in", [1, 1], F32)
    cc_out = nc.dram_tensor("cc_out", [1, NCORES], F32, addr_space="Shared")
    # Dummy collective buffers: a warm-up AllReduce at kernel start pays the
    # ncfw wakeup latency so the real one at the end doesn't.
    ccw_in = nc.dram_tensor("ccw_in", [1, 8], F32)
    ccw_out = nc.dram_tensor("ccw_out", [1, 8], F32, addr_space="Shared")

    out_ap = out_ext.ap()
    # key rows viewed as [p, t, d] with s = 32*p + t, flattened free dim.
    key_r = [
        key_ext.ap()[b].rearrange("(p t) d -> p (t d)", p=P) for b in range(BPC)
    ]

    with tile.TileContext(nc) as tc:
        with (
            tc.tile_pool(name="keys", bufs=KEY_BUFS) as kpool,
            tc.tile_pool(name="amr_scratch", bufs=4) as amrpool,
            tc.tile_pool(name="sq_psum", bufs=2, space="PSUM") as sqpool,
            tc.tile_pool(name="persist", bufs=1) as pp,
        ):
            # ---- setup: query broadcast, seq_lens, s-index, masks ----
            # q/sl ride the scalar (ACT) HWDGE ring; key loads own the sync
            # ring so they start flowing immediately.
            q_all = pp.tile([P, BPC * D], F32)
            nc.scalar.dma_start(out=q_all[0:1, :], in_=q_ext.ap()[:, :])
            sl_i = pp.tile([1, BPC], I32)
            nc.scalar.dma_start(out=sl_i[:, :], in_=sl_ext.ap()[:, :])

            # warm-up collective (result unused)
            warm = pp.tile([1, 8], F32)
            nc.vector.memset(warm[:, :], 0.0)
            nc.scalar.dma_start(out=ccw_in.ap()[:, :], in_=warm[:, :])
            nc.gpsimd.collective_compute(
                "AllReduce",
                ALU.add,
                replica_groups=[list(range(NCORES))],
                ins=[ccw_in.ap().opt()],
                outs=[ccw_out.ap().opt()],
            )

            nc.gpsimd.partition_broadcast(q_all[:, :], q_all[0:1, :])
            q_rep = [q_all[:, b * D : (b + 1) * D] for b in range(BPC)]

            sl_f = pp.tile([P, BPC], F32)
            nc.vector.tensor_copy(out=sl_f[0:1, :], in_=sl_i[:, :])
            nc.gpsimd.partition_broadcast(sl_f[:, :], sl_f[0:1, :])

            # s_idx[p, t] = 32*p + t  (the sequence position of scores[p, t])
            s_idx_i = pp.tile([P, TPB], I32)
            nc.gpsimd.iota(
                s_idx_i[:, :], pattern=[[1, TPB]], base=0, channel_multiplier=TPB
            )
            s_idx = pp.tile([P, TPB], F32)
            nc.vector.tensor_copy(out=s_idx[:, :], in_=s_idx_i[:, :])

            # masks depend only on s_idx/seq_lens: compute them up front
            masks = pp.tile([P, BPC * TPB], F32)
            for b in range(BPC):
                nc.vector.tensor_scalar(
                    out=masks[:, b * TPB : (b + 1) * TPB],
                    in0=s_idx[:, :],
                    scalar1=sl_f[:, b : b + 1],
                    scalar2=None,
                    op0=ALU.is_lt,
                )

            # ---- main streaming loop over key super-tiles ----
            scores = pp.tile([P, BPC * TPB], F32)
            ssqcols = pp.tile([P, NSQCOL], F32)

            def do_amr(kt, b, t, j):
                amr = amrpool.tile([P, D], F32, tag="amr")
                nc.vector.affine_mul_reduce(
                    out=amr[:, :],
                    accum_out=scores[:, b * TPB + t : b * TPB + t + 1],
                    in0=kt[:, j * D : (j + 1) * D],
                    in1=q_rep[b],
                    scale=1.0,
                    bias=0.0,
                )

            for b in range(BPC):
                for g in range(NG):
                    st = b * NG + g
                    kt = kpool.tile([P, SUB * D], F32, tag="key")
                    if st < NST - 1:
                        # one 2 MiB DMA; 16 KiB contiguous per partition.
                        # All key loads on the sync ring: routing them via
                        # the scalar (ACT) ring stalls issue behind the 2 us
                        # Square ops and makes the stream jagged (measured).
                        nc.sync.dma_start(
                            out=kt[:, :],
                            in_=key_r[b][:, g * SUB * D : (g + 1) * SUB * D],
                        )
                        for j in range(SUB):
                            do_amr(kt, b, g * SUB + j, j)
                        # ssq partials: square+accum, 2048 wide; out -> PSUM
                        for h in range(NSQ):
                            w = 2 * D
                            sq = sqpool.tile([P, w], F32, tag="sq")
                            nc.scalar.activation(
                                out=sq[:, :],
                                in_=kt[:, h * w : (h + 1) * w],
                                func=ACTF.Square,
                                accum_out=ssqcols[
                                    :, NSQ * st + h : NSQ * st + h + 1
                                ],
                            )
                    else:
                        # last super-tile: per-tile DMAs + 1024-wide squares
                        # so ssq lags the final DMA byte by only ~2 us
                        for j in range(SUB):
                            t = g * SUB + j
                            nc.sync.dma_start(
                                out=kt[:, j * D : (j + 1) * D],
                                in_=key_r[b][:, t * D : (t + 1) * D],
                            )
                            do_amr(kt, b, t, j)
                            sq = sqpool.tile([P, 2 * D], F32, tag="sq")
                            nc.scalar.activation(
                                out=sq[:, 0:D],
                                in_=kt[:, j * D : (j + 1) * D],
                                func=ACTF.Square,
                                accum_out=ssqcols[
                                    :, NSQ * st + j : NSQ * st + j + 1
                                ],
                            )

            # ---- dummy Exp+Ln right after the last square: forces the ACT
            # table loads to happen during the collective wait, not after.
            # Reading ssqcols' last column orders them after the last
            # accumulator read on the ACT queue. Ln LAST so the real Ln sees
            # no function-family transition (the framework reloads a table on
            # every transition, even if it was loaded before).
            dum = pp.tile([1, 2], F32)
            nc.scalar.activation(
                out=dum[:, 1:2],
                in_=ssqcols[0:1, NSQCOL - 1 : NSQCOL],
                func=ACTF.Exp,
                scale=-1e-6,
            )
            nc.scalar.activation(
                out=dum[:, 0:1], in_=ssqcols[0:1, NSQCOL - 1 : NSQCOL], func=ACTF.Ln
            )

            # ---- local ssq -> replicated scalar -> AllReduce ----
            ssq_p = pp.tile([P, 1], F32)
            nc.vector.tensor_reduce(
                out=ssq_p[:, :], in_=ssqcols[:, :], axis=AXL.X, op=ALU.add
            )
            nc.gpsimd.partition_all_reduce(
                ssq_p[:, :], ssq_p[:, :], P, ReduceOp.add
            )
            nc.scalar.dma_start(out=cc_in.ap()[:, :], in_=ssq_p[0:1, :])
            nc.gpsimd.collective_compute(
                "AllReduce",
                ALU.add,
                replica_groups=[list(range(NCORES))],
                ins=[cc_in.ap().opt()],
                outs=[cc_out.ap()[:, 0:1].opt()],
            )
            gssq = pp.tile([1, 1], F32)
            nc.scalar.dma_start(out=gssq[:, :], in_=cc_out.ap()[:, 0:1])

            # inv = 1/sqrt(gssq) = exp(-0.5 * ln(gssq)); tables preloaded.
            lng = pp.tile([1, 1], F32)
            nc.scalar.activation(out=lng[:, :], in_=gssq[:, :], func=ACTF.Ln)
            inv_rep = pp.tile([P, 1], F32)
            nc.scalar.activation(
                out=inv_rep[0:1, :], in_=lng[:, :], func=ACTF.Exp, scale=-0.5
            )
            nc.gpsimd.partition_broadcast(inv_rep[:, :], inv_rep[0:1, :])

            # ---- epilogue: masked softmax, renormalize, store ----
            e = pp.tile([P, BPC * TPB], F32)
            nc.scalar.activation(
                out=e[:, :], in_=scores[:, :], func=ACTF.Exp, scale=inv_rep[:, :]
            )
            em = pp.tile([P, BPC * TPB], F32)
            zc = pp.tile([P, BPC], F32)
            for b in range(BPC):
                sl_ = slice(b * TPB, (b + 1) * TPB)
                nc.vector.affine_mul_reduce(
                    out=em[:, sl_],
                    accum_out=zc[:, b : b + 1],
                    in0=e[:, sl_],
                    in1=masks[:, sl_],
                    scale=1.0,
                    bias=0.0,
                )
            nc.gpsimd.partition_all_reduce(zc[:, :], zc[:, :], P, ReduceOp.add)
            invz = pp.tile([P, BPC], F32)
            nc.vector.reciprocal(out=invz[:, :], in_=zc[:, :])

            o = pp.tile([P, BPC * TPB], F32)
            for b in range(BPC):
                sl_ = slice(b * TPB, (b + 1) * TPB)
                nc.vector.tensor_scalar(
                    out=o[:, sl_],
                    in0=em[:, sl_],
                    scalar1=invz[:, b : b + 1],
                    scalar2=PERTURB,
                    op0=ALU.mult,
                    op1=ALU.add,
                )
            # out[b, 32p+t] = o[p, b*32+t]: direct DMAs, 128 B per chunk,
            # alternating the two HWDGE rings so issue overlaps
            for b in range(BPC):
                dst = out_ap[b, :, 0].rearrange("(p t) -> p t", p=P)
                eng = nc.sync if b % 2 == 0 else nc.scalar
                eng.dma_start(out=dst, in_=o[:, b * TPB : (b + 1) * TPB])

    nc.compile()
    return nc


_NC_CACHE = None


def _get_nc():
    global _NC_CACHE
    if _NC_CACHE is None:
        _NC_CACHE = build()
    return _NC_CACHE


def make_in_maps(key, query, seq_lens):
    key = np.ascontiguousarray(np.asarray(key, dtype=np.float32))
    query = np.ascontiguousarray(np.asarray(query, dtype=np.float32))
    seq_lens = np.ascontiguousarray(np.asarray(seq_lens, dtype=np.int32))
    in_maps = []
    for c in range(NCORES):
        lo, hi = c * BPC, (c + 1) * BPC
        in_maps.append(
            {
                "key": key[lo:hi],
                "query": query[lo:hi].reshape(1, BPC * D),
                "seq_lens": seq_lens[lo:hi].reshape(1, BPC),
            }
        )
    return in_maps


def kernel(key, query, seq_lens, **run_kwargs):
    nc = _get_nc()
    in_maps = make_in_maps(key, query, seq_lens)
    res = run_bass_kernel_spmd(
        nc, in_maps, core_ids=list(range(NCORES)), **run_kwargs
    )
    outs = [res.results[c]["out"].reshape(BPC, S, 1) for c in range(NCORES)]
    full = np.concatenate(outs, axis=0).astype(np.float32)
    if run_kwargs:
        kernel.last_results = res  # expose profile info to test harness
    return full
